# revision 35
# baseline (speedup 1.0000x reference)
"""DeepSeekV3-style MoE layer on 8 Trainium2 NeuronCores.

Strategy (expert-parallel, host-side dispatch):
  - Host computes the sigmoid gate + top-2 routing (tiny: [8192,2048]@[2048,16]),
    gathers each expert's tokens. Experts are paired largest-with-smallest and
    sharded 2-per-core; per-slot capacities C0/C1 are the max count over the
    slot's 8 experts (exact, no rounding). The shared expert is data-parallel
    (1024 tokens per core).
  - Each core runs the same Bass/Tile program: 3 SwiGLU "units"
    (shared + 2 experts), weight-stationary matmuls at N<=512 in fp16
    (full PE rate, fast weight loads) with fp32 PSUM accumulation.
  - Phase 2 is h-major (stationary = w2 [128i,128h] tiles, moving = tokens):
    no ceil-128 token padding, exact token-column counts, output [H, n].
  - Startup: unit-0/it0 weights head the sync+scalar queues, x0 lands in
    four 256-column groups round-robined over all three DMA queues (gpsimd's
    share rides AHEAD of its w13 stream), and it0 runs 256-wide chunks so
    x delivery (~2.9us/group) outpaces PE consumption (~3.4us/chunk). PE
    stalls also reset the p-state clock ramp (0.65->1.2->2.4GHz over ~3us
    of continuous busy), so a stall-free startup pays twice.
    (All DMA is gated behind a fixed ~8.7us NEFF prologue; measured HW
    exec ~732us vs ~710us structural floor, 92.5% PE-active MFU.)
  - Gating scale is applied on-device during PSUM->SBUF evacuation via a
    host-replicated [128, cap] gate tile; host scatter-adds expert outputs
    back (transposing from [H, n]) and adds the shared output.

Layouts (host-prepared so every DMA is wide & contiguous):
  x*T   [16,128,n]        tokens transposed, h-tile major
  w1p   [3,11,128,2048]   phase-1 lhsT packs: [u][it][p=h%128][ht*128+j(=i%128)]
  w3p   same
  w2t   [3,11,128,2048]   w2 transposed: [u][it][p=i%128][h]
  gr*   [128,cap]         per-token gating scale, replicated over partitions
Outputs (h-major): ys [2048,1024], ye0 [2048,C0], ye1 [2048,C1] (fp32).
"""

import os
import sys

import numpy as np

if "/opt/trn_rl_repo" not in sys.path:
    sys.path.insert(0, "/opt/trn_rl_repo")

import concourse.bass as bass
import concourse.bacc as bacc
import concourse.mybir as mybir
import concourse.tile as tile
from concourse.bass_utils import run_bass_kernel_spmd

B, S, H, I, E, TOPK = 4, 2048, 2048, 1408, 16, 2
T = B * S               # 8192 tokens
NCORES = 8
NS = T // NCORES        # shared-expert tokens per core
HT, IT = H // 128, I // 128   # 16, 11
EPC = E // NCORES       # experts per core = 2

MM_MODE = os.environ.get("MOE_MM_MODE", "fp16")   # "fp16" | "f32r" | "bf16" | "f32"

LAST_RESULTS = None     # BassKernelResults of the last run (for test harness)

_PROGRAM_CACHE = {}
_PACK_CACHE = {}


def _sigmoid(x):
    out = np.empty_like(x)
    np.negative(x, out=out)
    np.exp(out, out=out)
    out += 1.0
    np.reciprocal(out, out=out)
    return out


def _chunks(n):
    """Split n into chunks <=512, all >=256 when n allows (f32r matmul runs
    at 1/4 rate below a 256-wide moving dim)."""
    out, rem = [], n
    while rem > 0:
        if rem <= 512:
            c = rem
        elif rem >= 768:
            c = 512
        else:  # rem in (512, 768): split so both pieces are >= 256
            c = rem - 256
        out.append(c)
        rem -= c
    return out


def _ramp_chunks(n):
    """Startup chunk widths for unit 0 / it 0, sized so the PE starts once
    ~2MB has landed and then NEVER stalls (x delivery outpaces consumption).
    Stall-free matters doubly: every PE idle gap resets the p-state clock
    ramp (0.65->2.4GHz over 3us of continuous busy)."""
    if n >= 1024:
        return [128, 128, 256, 256, 256] + _chunks(n - 1024)
    return _chunks(n)


def _build_program(caps, mode):
    """caps = (C0, C1): exact token capacity of the two local expert slots."""
    key = (caps, mode)
    if key in _PROGRAM_CACHE:
        return _PROGRAM_CACHE[key]

    if mode == "bf16":
        in_dt = mybir.dt.bfloat16
    elif mode == "fp16":
        in_dt = mybir.dt.float16
    elif mode == "f32r":
        in_dt = mybir.dt.float32r
    else:
        in_dt = mybir.dt.float32
    f32 = mybir.dt.float32
    n_units = [NS, caps[0], caps[1]]
    CW = max(n_units)       # tile width shared by xt/g tags

    nc = bacc.Bacc("TRN2", target_bir_lowering=False, debug=False)

    xT = [nc.dram_tensor(f"x{u}T", [HT, 128, n_units[u]], in_dt,
                         kind="ExternalInput").ap() for u in range(3)]
    w1p = nc.dram_tensor("w1p", [3, IT, 128, H], in_dt, kind="ExternalInput").ap()
    w3p = nc.dram_tensor("w3p", [3, IT, 128, H], in_dt, kind="ExternalInput").ap()
    w2t = nc.dram_tensor("w2t", [3, IT, 128, H], in_dt, kind="ExternalInput").ap()
    gr = [None] + [nc.dram_tensor(f"gr{u}", [128, n_units[u]], f32,
                                  kind="ExternalInput").ap() for u in (1, 2)]
    yo = [nc.dram_tensor(["ys", "ye0", "ye1"][u], [H, n_units[u]], f32,
                         kind="ExternalOutput").ap() for u in range(3)]

    # DMA-queue plan (one HW queue per engine, FIFO): sync carries only the
    # activation loads, scalar only the output writes (plus half of the x0
    # race), gpsimd all weight streams. Emission order = descriptor order,
    # so prefetches are hoisted ahead of the compute that needs them.
    with tile.TileContext(nc) as tc:
        with (
            tc.tile_pool(name="xt", bufs=HT) as xt_pool,
            tc.tile_pool(name="g", bufs=IT + 1) as g_pool,
            tc.tile_pool(name="w13", bufs=6) as w13_pool,
            tc.tile_pool(name="w2", bufs=IT + 1) as w2_pool,
            tc.tile_pool(name="grb", bufs=2) as gr_pool,
            tc.tile_pool(name="ot", bufs=4) as out_pool,
            tc.tile_pool(name="ps", bufs=8, space="PSUM") as ps_pool,
        ):
            def load_xt(u):
                n_u = n_units[u]
                xts = [xt_pool.tile([128, CW], in_dt, tag="xt",
                                    name=f"xt{u}_{ht}") for ht in range(HT)]
                if u == 0:
                    # racing the kernel start: three queues, landing column
                    # groups that match the it0 ramp chunks [128,384,512,...]
                    # so each chunk's chains can begin while the rest streams
                    bounds = [0]
                    for w in _ramp_chunks(n_u):
                        bounds.append(bounds[-1] + w)
                    # merge the tail groups (beyond the ramp) into one DMA
                    if len(bounds) > 6:
                        bounds = bounds[:6] + [n_u]
                    # gpsimd carries a third of x AHEAD of its w13 stream,
                    # giving x strict priority on all three DMA queues
                    engs = [nc.sync, nc.scalar, nc.gpsimd]
                    for gi, (g0, g1) in enumerate(zip(bounds[:-1],
                                                      bounds[1:])):
                        for ht in range(HT):
                            engs[ht % 3].dma_start(out=xts[ht][:, g0:g1],
                                                   in_=xT[u][ht][:, g0:g1])
                    # it1 weights follow the x race on sync/scalar: they
                    # land right when it0's chains finish, and keep gpsimd
                    # free to deliver it2+ during it0
                    nc.sync.dma_start(out=it1_w[0][:], in_=w1p[0, 1])
                    nc.scalar.dma_start(out=it1_w[1][:], in_=w3p[0, 1])

                else:
                    for ht in range(HT):
                        nc.sync.dma_start(out=xts[ht][:, :n_u], in_=xT[u][ht])
                return xts

            # unit-0 it0 weights head the sync/scalar queues (before the x
            # race) so the first phase-1 chain can start ~4us after the
            # NEFF prologue; the x groups follow on three queues.
            w1t00 = w13_pool.tile([128, H], in_dt, tag="w13", name="w1t0_0")
            w3t00 = w13_pool.tile([128, H], in_dt, tag="w13", name="w3t0_0")
            nc.sync.dma_start(out=w1t00[:], in_=w1p[0, 0])
            nc.scalar.dma_start(out=w3t00[:], in_=w3p[0, 0])
            it1_w = (w13_pool.tile([128, H], in_dt, tag="w13", name="w1t0_1"),
                     w13_pool.tile([128, H], in_dt, tag="w13", name="w3t0_1"))
            xts = load_xt(0)
            for u in range(3):
                n_u = n_units[u]

                # ---- weight-stream emission (gpsimd): w13 it0/it1 first,
                # then this unit's gating tile, then the rest of w13, then
                # (after it10) the full w2 tile set for phase 2 (it lands
                # during this unit's phase 1; slot-waits pace the queue)
                w13s = []
                for it in range(IT):
                    if u == 0 and it == 0:
                        w13s.append((w1t00, w3t00))
                        continue
                    if u == 0 and it == 1:
                        w13s.append(it1_w)
                        continue

                    w1t = w13_pool.tile([128, H], in_dt, tag="w13",
                                        name=f"w1t{u}_{it}")
                    w3t = w13_pool.tile([128, H], in_dt, tag="w13",
                                        name=f"w3t{u}_{it}")
                    w13s.append((w1t, w3t))
                    nc.gpsimd.dma_start(out=w1t[:], in_=w1p[u, it])
                    nc.gpsimd.dma_start(out=w3t[:], in_=w3p[u, it])
                    if it == 5 and u > 0:
                        grt = gr_pool.tile([128, n_u], f32, tag="grb",
                                           name=f"grt{u}")
                        nc.gpsimd.dma_start(out=grt[:], in_=gr[u])
                w2s = []
                for it in range(IT):
                    w2tile = w2_pool.tile([128, H], in_dt, tag="w2",
                                          name=f"w2_{u}_{it}")
                    nc.gpsimd.dma_start(out=w2tile[:], in_=w2t[u, it])
                    w2s.append(w2tile)

                # ---- phase 1: G^T[i, t] = silu(W1 xT) * (W3 xT) ----
                gts = []
                for it in range(IT):
                    w1t, w3t = w13s[it]
                    gt = g_pool.tile([128, CW], in_dt, tag="g", name=f"g{u}_{it}")
                    gts.append(gt)
                    c0 = 0
                    cl = _ramp_chunks(n_u) if (u == 0 and it == 0) \
                        else _chunks(n_u)
                    for w in cl:
                        ps1 = ps_pool.tile([128, 512], f32, tag="ps",
                                           name=f"ps1_{u}_{it}_{c0}")
                        ps3 = ps_pool.tile([128, 512], f32, tag="ps",
                                           name=f"ps3_{u}_{it}_{c0}")
                        # interleave the w1/w3 accumulation chains: each
                        # chain's end-of-chain array drain overlaps the
                        # other chain's stream (~65ns/chain boundary saved)
                        for ht in range(HT):
                            nc.tensor.matmul(
                                ps1[:, :w], w1t[:, ht * 128:(ht + 1) * 128],
                                xts[ht][:, c0:c0 + w],
                                start=(ht == 0), stop=(ht == HT - 1))
                            nc.tensor.matmul(
                                ps3[:, :w], w3t[:, ht * 128:(ht + 1) * 128],
                                xts[ht][:, c0:c0 + w],
                                start=(ht == 0), stop=(ht == HT - 1))
                        # silu(h1)*h3 = sigmoid(h1)*h1*h3 (Silu not in CoreSim)
                        gsl = gt[:, c0:c0 + w]
                        nc.scalar.activation(gsl, ps1[:, :w],
                                             mybir.ActivationFunctionType.Sigmoid)
                        nc.vector.tensor_mul(gsl, gsl, ps1[:, :w])
                        nc.vector.tensor_mul(gsl, gsl, ps3[:, :w])
                        c0 += w

                # next unit's activations stream during phase 2
                if u < 2:
                    next_xts = load_xt(u + 1)

                # ---- phase 2 (h-major): Y[h, t] = W2^T.T @ G^T, +gating ----
                # stationary = w2 [128i,128h] slices, moving = token columns;
                # exact token counts (no ceil-128 padding), output [H, n_u].
                p2c = _chunks(n_u)
                c0 = 0
                for ci, w in enumerate(p2c):
                    last_chunk = (u == 2 and ci == len(p2c) - 1)
                    for hp in range(HT // 2):
                        # two ht chains interleaved into two PSUM banks so
                        # each chain's drain overlaps the other's stream
                        hts = (2 * hp, 2 * hp + 1)
                        pss = [ps_pool.tile([128, 512], f32, tag="ps",
                                            name=f"ps2_{u}_{c0}_{ht}")
                               for ht in hts]
                        for it in range(IT):
                            for ps, ht in zip(pss, hts):
                                nc.tensor.matmul(
                                    ps[:, :w],
                                    w2s[it][:, ht * 128:(ht + 1) * 128],
                                    gts[it][:, c0:c0 + w],
                                    start=(it == 0), stop=(it == IT - 1))
                        for ps, ht in zip(pss, hts):
                            ot = out_pool.tile([128, 512], f32, tag="ot",
                                               name=f"ot{u}_{c0}_{ht}")
                            if u == 0:
                                nc.vector.tensor_copy(ot[:, :w], ps[:, :w])
                            else:
                                nc.vector.tensor_mul(ot[:, :w], ps[:, :w],
                                                     grt[:, c0:c0 + w])
                            # the very last chunk's writes alternate with
                            # the (by then idle) sync queue: halves the
                            # end-of-kernel write drain
                            weng = nc.sync if (last_chunk and ht % 2) \
                                else nc.scalar
                            weng.dma_start(
                                out=yo[u][ht * 128:(ht + 1) * 128,
                                          c0:c0 + w],
                                in_=ot[:, :w])
                    c0 += w
                if u < 2:
                    xts = next_xts

    nc.compile()
    _PROGRAM_CACHE[key] = nc
    return nc


def _np_dt(mode):
    if mode == "bf16":
        import ml_dtypes
        return np.dtype(ml_dtypes.bfloat16)
    if mode == "fp16":
        return np.dtype(np.float16)
    return np.dtype(np.float32)


def _pack_w13(w, dt):
    """[I,H] -> [IT,128,H] with [it, p, ht*128+j] = w[it*128+j, ht*128+p]."""
    a = np.ascontiguousarray(
        w.reshape(IT, 128, HT, 128).transpose(0, 3, 2, 1), dtype=dt)
    return a.reshape(IT, 128, H)


def _pack_w2(w, dt):
    """[H,I] -> [IT,128,H]  (= w.T tiled along I)."""
    return np.ascontiguousarray(w.T.reshape(IT, 128, H), dtype=dt)


def _pack_all_weights(shared_w1, shared_w3, shared_w2, w1, w3, w2, mode):
    key = (id(w1), id(w2), id(w3), mode)
    if _PACK_CACHE.get("key") == key:
        return _PACK_CACHE["val"]
    dt = _np_dt(mode)
    p1 = [_pack_w13(shared_w1, dt)] + [_pack_w13(w1[e], dt) for e in range(E)]
    p3 = [_pack_w13(shared_w3, dt)] + [_pack_w13(w3[e], dt) for e in range(E)]
    p2 = [_pack_w2(shared_w2, dt)] + [_pack_w2(w2[e], dt) for e in range(E)]
    val = (p1, p3, p2)
    _PACK_CACHE["key"] = key
    _PACK_CACHE["val"] = val
    return val


def _prepare(hidden_states, gate_w, bias, shared_w1, shared_w3, shared_w2,
             w1, w3, w2, mode):
    """Host routing + per-core input maps. Returns (nc, in_maps, meta)."""
    x = np.ascontiguousarray(hidden_states.reshape(T, H), dtype=np.float32)

    scores = _sigmoid(x @ gate_w.T.astype(np.float32))
    routing = scores + bias.astype(np.float32)[None, :]
    topk = np.argsort(-routing, axis=1, kind="stable")[:, :TOPK]
    sel = np.take_along_axis(scores, topk, axis=1)
    gating = (sel / sel.sum(axis=1, keepdims=True)).astype(np.float32)

    flat_t = np.repeat(np.arange(T), TOPK)
    flat_e = topk.ravel()
    flat_g = gating.ravel()
    order = np.argsort(flat_e, kind="stable")
    flat_t, flat_g = flat_t[order], flat_g[order]
    counts = np.bincount(flat_e, minlength=E)
    offs = np.zeros(E + 1, np.int64)
    np.cumsum(counts, out=offs[1:])

    # pair largest with smallest: slot0 = rank c, slot1 = rank 15-c
    # (minimizes C0+C1 = c(1)+c(9), which is optimal for a 2-slot SPMD plan)
    rank = np.argsort(-counts, kind="stable")
    slot_experts = [(int(rank[c]), int(rank[E - 1 - c])) for c in range(NCORES)]
    C0 = max(1, int(counts[rank[0]]))
    C1 = max(1, int(counts[rank[NCORES]]))
    caps = (C0, C1)

    nc = _build_program(caps, mode)
    dt = _np_dt(mode)

    p1, p3, p2 = _pack_all_weights(shared_w1, shared_w3, shared_w2,
                                   w1, w3, w2, mode)
    xc = x.astype(dt, copy=False)

    tok_ids = []
    in_maps = []
    for c in range(NCORES):
        im = {"x0T": np.ascontiguousarray(
            xc[c * NS:(c + 1) * NS].T).reshape(HT, 128, NS)}
        ids_pair = []
        for j, e in enumerate(slot_experts[c]):
            Cj = caps[j]
            ids = flat_t[offs[e]:offs[e + 1]]
            ids_pair.append(ids)
            n = len(ids)
            xg = np.zeros((Cj, H), dt)
            xg[:n] = xc[ids]
            im[f"x{j + 1}T"] = np.ascontiguousarray(xg.T).reshape(HT, 128, Cj)
            grow = np.zeros((Cj,), np.float32)
            grow[:n] = flat_g[offs[e]:offs[e + 1]]
            im[f"gr{j + 1}"] = np.ascontiguousarray(
                np.broadcast_to(grow, (128, Cj)))
        e0, e1 = slot_experts[c]
        im["w1p"] = np.stack([p1[0], p1[1 + e0], p1[1 + e1]])
        im["w3p"] = np.stack([p3[0], p3[1 + e0], p3[1 + e1]])
        im["w2t"] = np.stack([p2[0], p2[1 + e0], p2[1 + e1]])
        tok_ids.append(ids_pair)
        in_maps.append(im)

    meta = {"counts": counts, "tok_ids": tok_ids, "slot_experts": slot_experts,
            "caps": caps, "shape": hidden_states.shape}
    return nc, in_maps, meta


def _combine(results, meta):
    out = np.empty((T, H), np.float32)
    for c in range(NCORES):
        out[c * NS:(c + 1) * NS] = results[c]["ys"].T
    for c in range(NCORES):
        for j in range(EPC):
            ids = meta["tok_ids"][c][j]
            out[ids] += results[c][f"ye{j}"][:, :len(ids)].T
    return out.reshape(meta["shape"])


def kernel(hidden_states, gate_w, bias, shared_w1, shared_w3, shared_w2,
           w1, w3, w2):
    args = [np.asarray(a) for a in (hidden_states, gate_w, bias, shared_w1,
                                    shared_w3, shared_w2, w1, w3, w2)]
    nc, in_maps, meta = _prepare(*args, MM_MODE)
    global LAST_RESULTS
    try:
        res = run_bass_kernel_spmd(nc, in_maps, list(range(NCORES)))
    except Exception:
        # transient NRT device errors happen; one retry clears them
        res = run_bass_kernel_spmd(nc, in_maps, list(range(NCORES)))
    LAST_RESULTS = res
    return _combine(res.results, meta)


# revision 37
# speedup vs baseline: 1.0007x; 1.0007x over previous
"""DeepSeekV3-style MoE layer on 8 Trainium2 NeuronCores.

Strategy (expert-parallel, host-side dispatch):
  - Host computes the sigmoid gate + top-2 routing (tiny: [8192,2048]@[2048,16]),
    gathers each expert's tokens. Experts are paired largest-with-smallest and
    sharded 2-per-core; per-slot capacities C0/C1 are the max count over the
    slot's 8 experts (exact, no rounding). The shared expert is data-parallel
    (1024 tokens per core).
  - Each core runs the same Bass/Tile program: 3 SwiGLU "units"
    (shared + 2 experts), weight-stationary matmuls at N<=512 in fp16
    (full PE rate, fast weight loads) with fp32 PSUM accumulation.
  - Phase 2 is h-major (stationary = w2 [128i,128h] tiles, moving = tokens):
    no ceil-128 token padding, exact token-column counts, output [H, n].
  - Startup: unit-0/it0 weights head the sync+scalar queues, x0 lands in
    four 256-column groups round-robined over all three DMA queues (gpsimd's
    share rides AHEAD of its w13 stream), and it0 runs 256-wide chunks so
    x delivery (~2.9us/group) outpaces PE consumption (~3.4us/chunk). PE
    stalls also reset the p-state clock ramp (0.65->1.2->2.4GHz over ~3us
    of continuous busy), so a stall-free startup pays twice.
    (All DMA is gated behind a fixed ~8.7us NEFF prologue; measured HW
    exec ~732us vs ~710us structural floor, 92.5% PE-active MFU.)
  - Gating scale is applied on-device during PSUM->SBUF evacuation via a
    host-replicated [128, cap] gate tile; host scatter-adds expert outputs
    back (transposing from [H, n]) and adds the shared output.

Layouts (host-prepared so every DMA is wide & contiguous):
  x*T   [16,128,n]        tokens transposed, h-tile major
  w1p   [3,11,128,2048]   phase-1 lhsT packs: [u][it][p=h%128][ht*128+j(=i%128)]
  w3p   same
  w2t   [3,11,128,2048]   w2 transposed: [u][it][p=i%128][h]
  gr*   [128,cap]         per-token gating scale, replicated over partitions
Outputs (h-major): ys [2048,1024], ye0 [2048,C0], ye1 [2048,C1] (fp32).
"""

import os
import sys

import numpy as np

if "/opt/trn_rl_repo" not in sys.path:
    sys.path.insert(0, "/opt/trn_rl_repo")

import concourse.bass as bass
import concourse.bacc as bacc
import concourse.mybir as mybir
import concourse.tile as tile
from concourse.bass_utils import run_bass_kernel_spmd

B, S, H, I, E, TOPK = 4, 2048, 2048, 1408, 16, 2
T = B * S               # 8192 tokens
NCORES = 8
NS = T // NCORES        # shared-expert tokens per core
HT, IT = H // 128, I // 128   # 16, 11
EPC = E // NCORES       # experts per core = 2

MM_MODE = os.environ.get("MOE_MM_MODE", "fp16")   # "fp16" | "f32r" | "bf16" | "f32"

LAST_RESULTS = None     # BassKernelResults of the last run (for test harness)

_PROGRAM_CACHE = {}
_PACK_CACHE = {}


def _sigmoid(x):
    out = np.empty_like(x)
    np.negative(x, out=out)
    np.exp(out, out=out)
    out += 1.0
    np.reciprocal(out, out=out)
    return out


def _chunks(n):
    """Split n into chunks <=512, all >=256 when n allows (f32r matmul runs
    at 1/4 rate below a 256-wide moving dim)."""
    out, rem = [], n
    while rem > 0:
        if rem <= 512:
            c = rem
        elif rem >= 768:
            c = 512
        else:  # rem in (512, 768): split so both pieces are >= 256
            c = rem - 256
        out.append(c)
        rem -= c
    return out


def _ramp_chunks(n):
    """Startup chunk widths for unit 0 / it 0, sized so the PE starts once
    ~2MB has landed and then NEVER stalls (x delivery outpaces consumption).
    Stall-free matters doubly: every PE idle gap resets the p-state clock
    ramp (0.65->2.4GHz over 3us of continuous busy)."""
    if n >= 1024:
        return [128, 128, 256, 256, 256] + _chunks(n - 1024)
    return _chunks(n)


def _build_program(caps, mode):
    """caps = (C0, C1): exact token capacity of the two local expert slots."""
    key = (caps, mode)
    if key in _PROGRAM_CACHE:
        return _PROGRAM_CACHE[key]

    if mode == "bf16":
        in_dt = mybir.dt.bfloat16
    elif mode == "fp16":
        in_dt = mybir.dt.float16
    elif mode == "f32r":
        in_dt = mybir.dt.float32r
    else:
        in_dt = mybir.dt.float32
    f32 = mybir.dt.float32
    n_units = [NS, caps[0], caps[1]]
    CW = max(n_units)       # tile width shared by xt/g tags

    nc = bacc.Bacc("TRN2", target_bir_lowering=False, debug=False)

    xT = [nc.dram_tensor(f"x{u}T", [HT, 128, n_units[u]], in_dt,
                         kind="ExternalInput").ap() for u in range(3)]
    w1p = nc.dram_tensor("w1p", [3, IT, 128, H], in_dt, kind="ExternalInput").ap()
    w3p = nc.dram_tensor("w3p", [3, IT, 128, H], in_dt, kind="ExternalInput").ap()
    w2t = nc.dram_tensor("w2t", [3, IT, 128, H], in_dt, kind="ExternalInput").ap()
    gr = [None] + [nc.dram_tensor(f"gr{u}", [128, n_units[u]], f32,
                                  kind="ExternalInput").ap() for u in (1, 2)]
    yo = [nc.dram_tensor(["ys", "ye0", "ye1"][u], [H, n_units[u]], f32,
                         kind="ExternalOutput").ap() for u in range(3)]

    # DMA-queue plan (one HW queue per engine, FIFO): sync carries only the
    # activation loads, scalar only the output writes (plus half of the x0
    # race), gpsimd all weight streams. Emission order = descriptor order,
    # so prefetches are hoisted ahead of the compute that needs them.
    with tile.TileContext(nc) as tc:
        with (
            tc.tile_pool(name="xt", bufs=HT) as xt_pool,
            tc.tile_pool(name="g", bufs=IT + 1) as g_pool,
            tc.tile_pool(name="w13", bufs=6) as w13_pool,
            tc.tile_pool(name="w2", bufs=IT + 1) as w2_pool,
            tc.tile_pool(name="grb", bufs=2) as gr_pool,
            tc.tile_pool(name="ot", bufs=4) as out_pool,
            tc.tile_pool(name="ps", bufs=8, space="PSUM") as ps_pool,
        ):
            def load_xt(u):
                n_u = n_units[u]
                xts = [xt_pool.tile([128, CW], in_dt, tag="xt",
                                    name=f"xt{u}_{ht}") for ht in range(HT)]
                if u == 0:
                    # racing the kernel start: three queues, landing column
                    # groups that match the it0 ramp chunks [128,384,512,...]
                    # so each chunk's chains can begin while the rest streams
                    bounds = [0]
                    for w in _ramp_chunks(n_u):
                        bounds.append(bounds[-1] + w)
                    # merge the tail groups (beyond the ramp) into one DMA
                    if len(bounds) > 6:
                        bounds = bounds[:6] + [n_u]
                    # gpsimd carries a third of x AHEAD of its w13 stream,
                    # giving x strict priority on all three DMA queues
                    engs = [nc.sync, nc.scalar, nc.gpsimd]
                    for gi, (g0, g1) in enumerate(zip(bounds[:-1],
                                                      bounds[1:])):
                        for ht in range(HT):
                            engs[ht % 3].dma_start(out=xts[ht][:, g0:g1],
                                                   in_=xT[u][ht][:, g0:g1])
                    # it1 weights follow the x race on sync/scalar: they
                    # land right when it0's chains finish, and keep gpsimd
                    # free to deliver it2+ during it0
                    nc.sync.dma_start(out=it1_w[0][:], in_=w1p[0, 1])
                    nc.scalar.dma_start(out=it1_w[1][:], in_=w3p[0, 1])

                else:
                    for ht in range(HT):
                        nc.sync.dma_start(out=xts[ht][:, :n_u], in_=xT[u][ht])
                return xts

            # unit-0 it0 weights head the sync/scalar queues (before the x
            # race) so the first phase-1 chain can start ~4us after the
            # NEFF prologue; the x groups follow on three queues.
            w1t00 = w13_pool.tile([128, H], in_dt, tag="w13", name="w1t0_0")
            w3t00 = w13_pool.tile([128, H], in_dt, tag="w13", name="w3t0_0")
            nc.sync.dma_start(out=w1t00[:], in_=w1p[0, 0])
            nc.scalar.dma_start(out=w3t00[:], in_=w3p[0, 0])
            it1_w = (w13_pool.tile([128, H], in_dt, tag="w13", name="w1t0_1"),
                     w13_pool.tile([128, H], in_dt, tag="w13", name="w3t0_1"))
            xts = load_xt(0)
            for u in range(3):
                n_u = n_units[u]

                # ---- weight-stream emission (gpsimd): w13 it0/it1 first,
                # then this unit's gating tile, then the rest of w13, then
                # (after it10) the full w2 tile set for phase 2 (it lands
                # during this unit's phase 1; slot-waits pace the queue)
                w13s = []
                for it in range(IT):
                    if u == 0 and it == 0:
                        w13s.append((w1t00, w3t00))
                        continue
                    if u == 0 and it == 1:
                        w13s.append(it1_w)
                        continue

                    w1t = w13_pool.tile([128, H], in_dt, tag="w13",
                                        name=f"w1t{u}_{it}")
                    w3t = w13_pool.tile([128, H], in_dt, tag="w13",
                                        name=f"w3t{u}_{it}")
                    w13s.append((w1t, w3t))
                    nc.gpsimd.dma_start(out=w1t[:], in_=w1p[u, it])
                    nc.gpsimd.dma_start(out=w3t[:], in_=w3p[u, it])
                    if it == 5 and u > 0:
                        grt = gr_pool.tile([128, n_u], f32, tag="grb",
                                           name=f"grt{u}")
                        nc.gpsimd.dma_start(out=grt[:], in_=gr[u])
                w2s = []
                for it in range(IT):
                    w2tile = w2_pool.tile([128, H], in_dt, tag="w2",
                                          name=f"w2_{u}_{it}")
                    nc.gpsimd.dma_start(out=w2tile[:], in_=w2t[u, it])
                    w2s.append(w2tile)

                # ---- phase 1: G^T[i, t] = silu(W1 xT) * (W3 xT) ----
                gts = []
                for it in range(IT):
                    w1t, w3t = w13s[it]
                    gt = g_pool.tile([128, CW], in_dt, tag="g", name=f"g{u}_{it}")
                    gts.append(gt)
                    c0 = 0
                    cl = _ramp_chunks(n_u) if (u == 0 and it == 0) \
                        else _chunks(n_u)
                    for w in cl:
                        ps1 = ps_pool.tile([128, 512], f32, tag="ps",
                                           name=f"ps1_{u}_{it}_{c0}")
                        ps3 = ps_pool.tile([128, 512], f32, tag="ps",
                                           name=f"ps3_{u}_{it}_{c0}")
                        for ht in range(HT):
                            nc.tensor.matmul(
                                ps1[:, :w], w1t[:, ht * 128:(ht + 1) * 128],
                                xts[ht][:, c0:c0 + w],
                                start=(ht == 0), stop=(ht == HT - 1))
                        for ht in range(HT):
                            nc.tensor.matmul(
                                ps3[:, :w], w3t[:, ht * 128:(ht + 1) * 128],
                                xts[ht][:, c0:c0 + w],
                                start=(ht == 0), stop=(ht == HT - 1))
                        # silu(h1)*h3 = sigmoid(h1)*h1*h3 (Silu not in CoreSim)
                        gsl = gt[:, c0:c0 + w]
                        nc.scalar.activation(gsl, ps1[:, :w],
                                             mybir.ActivationFunctionType.Sigmoid)
                        nc.vector.tensor_mul(gsl, gsl, ps1[:, :w])
                        nc.vector.tensor_mul(gsl, gsl, ps3[:, :w])
                        c0 += w

                # next unit's activations stream during phase 2
                if u < 2:
                    next_xts = load_xt(u + 1)

                # ---- phase 2 (h-major): Y[h, t] = W2^T.T @ G^T, +gating ----
                # stationary = w2 [128i,128h] slices, moving = token columns;
                # exact token counts (no ceil-128 padding), output [H, n_u].
                p2c = _chunks(n_u)
                c0 = 0
                for ci, w in enumerate(p2c):
                    last_chunk = (u == 2 and ci == len(p2c) - 1)
                    for ht in range(HT):
                        ps = ps_pool.tile([128, 512], f32, tag="ps",
                                          name=f"ps2_{u}_{c0}_{ht}")
                        for it in range(IT):
                            nc.tensor.matmul(
                                ps[:, :w],
                                w2s[it][:, ht * 128:(ht + 1) * 128],
                                gts[it][:, c0:c0 + w],
                                start=(it == 0), stop=(it == IT - 1))
                        ot = out_pool.tile([128, 512], f32, tag="ot",
                                           name=f"ot{u}_{c0}_{ht}")
                        if u == 0:
                            nc.vector.tensor_copy(ot[:, :w], ps[:, :w])
                        else:
                            nc.vector.tensor_mul(ot[:, :w], ps[:, :w],
                                                 grt[:, c0:c0 + w])
                        # the very last chunk's writes alternate with the
                        # (by then idle) sync queue to halve the end drain
                        weng = nc.sync if (last_chunk and ht % 2) else nc.scalar
                        weng.dma_start(
                            out=yo[u][ht * 128:(ht + 1) * 128, c0:c0 + w],
                            in_=ot[:, :w])
                    c0 += w
                if u < 2:
                    xts = next_xts

    nc.compile()
    _PROGRAM_CACHE[key] = nc
    return nc


def _np_dt(mode):
    if mode == "bf16":
        import ml_dtypes
        return np.dtype(ml_dtypes.bfloat16)
    if mode == "fp16":
        return np.dtype(np.float16)
    return np.dtype(np.float32)


def _pack_w13(w, dt):
    """[I,H] -> [IT,128,H] with [it, p, ht*128+j] = w[it*128+j, ht*128+p]."""
    a = np.ascontiguousarray(
        w.reshape(IT, 128, HT, 128).transpose(0, 3, 2, 1), dtype=dt)
    return a.reshape(IT, 128, H)


def _pack_w2(w, dt):
    """[H,I] -> [IT,128,H]  (= w.T tiled along I)."""
    return np.ascontiguousarray(w.T.reshape(IT, 128, H), dtype=dt)


def _pack_all_weights(shared_w1, shared_w3, shared_w2, w1, w3, w2, mode):
    key = (id(w1), id(w2), id(w3), mode)
    if _PACK_CACHE.get("key") == key:
        return _PACK_CACHE["val"]
    dt = _np_dt(mode)
    p1 = [_pack_w13(shared_w1, dt)] + [_pack_w13(w1[e], dt) for e in range(E)]
    p3 = [_pack_w13(shared_w3, dt)] + [_pack_w13(w3[e], dt) for e in range(E)]
    p2 = [_pack_w2(shared_w2, dt)] + [_pack_w2(w2[e], dt) for e in range(E)]
    val = (p1, p3, p2)
    _PACK_CACHE["key"] = key
    _PACK_CACHE["val"] = val
    return val


def _prepare(hidden_states, gate_w, bias, shared_w1, shared_w3, shared_w2,
             w1, w3, w2, mode):
    """Host routing + per-core input maps. Returns (nc, in_maps, meta)."""
    x = np.ascontiguousarray(hidden_states.reshape(T, H), dtype=np.float32)

    scores = _sigmoid(x @ gate_w.T.astype(np.float32))
    routing = scores + bias.astype(np.float32)[None, :]
    topk = np.argsort(-routing, axis=1, kind="stable")[:, :TOPK]
    sel = np.take_along_axis(scores, topk, axis=1)
    gating = (sel / sel.sum(axis=1, keepdims=True)).astype(np.float32)

    flat_t = np.repeat(np.arange(T), TOPK)
    flat_e = topk.ravel()
    flat_g = gating.ravel()
    order = np.argsort(flat_e, kind="stable")
    flat_t, flat_g = flat_t[order], flat_g[order]
    counts = np.bincount(flat_e, minlength=E)
    offs = np.zeros(E + 1, np.int64)
    np.cumsum(counts, out=offs[1:])

    # pair largest with smallest: slot0 = rank c, slot1 = rank 15-c
    # (minimizes C0+C1 = c(1)+c(9), which is optimal for a 2-slot SPMD plan)
    rank = np.argsort(-counts, kind="stable")
    slot_experts = [(int(rank[c]), int(rank[E - 1 - c])) for c in range(NCORES)]
    C0 = max(1, int(counts[rank[0]]))
    C1 = max(1, int(counts[rank[NCORES]]))
    caps = (C0, C1)

    nc = _build_program(caps, mode)
    dt = _np_dt(mode)

    p1, p3, p2 = _pack_all_weights(shared_w1, shared_w3, shared_w2,
                                   w1, w3, w2, mode)
    xc = x.astype(dt, copy=False)

    tok_ids = []
    in_maps = []
    for c in range(NCORES):
        im = {"x0T": np.ascontiguousarray(
            xc[c * NS:(c + 1) * NS].T).reshape(HT, 128, NS)}
        ids_pair = []
        for j, e in enumerate(slot_experts[c]):
            Cj = caps[j]
            ids = flat_t[offs[e]:offs[e + 1]]
            ids_pair.append(ids)
            n = len(ids)
            xg = np.zeros((Cj, H), dt)
            xg[:n] = xc[ids]
            im[f"x{j + 1}T"] = np.ascontiguousarray(xg.T).reshape(HT, 128, Cj)
            grow = np.zeros((Cj,), np.float32)
            grow[:n] = flat_g[offs[e]:offs[e + 1]]
            im[f"gr{j + 1}"] = np.ascontiguousarray(
                np.broadcast_to(grow, (128, Cj)))
        e0, e1 = slot_experts[c]
        im["w1p"] = np.stack([p1[0], p1[1 + e0], p1[1 + e1]])
        im["w3p"] = np.stack([p3[0], p3[1 + e0], p3[1 + e1]])
        im["w2t"] = np.stack([p2[0], p2[1 + e0], p2[1 + e1]])
        tok_ids.append(ids_pair)
        in_maps.append(im)

    meta = {"counts": counts, "tok_ids": tok_ids, "slot_experts": slot_experts,
            "caps": caps, "shape": hidden_states.shape}
    return nc, in_maps, meta


def _combine(results, meta):
    out = np.empty((T, H), np.float32)
    for c in range(NCORES):
        out[c * NS:(c + 1) * NS] = results[c]["ys"].T
    for c in range(NCORES):
        for j in range(EPC):
            ids = meta["tok_ids"][c][j]
            out[ids] += results[c][f"ye{j}"][:, :len(ids)].T
    return out.reshape(meta["shape"])


def kernel(hidden_states, gate_w, bias, shared_w1, shared_w3, shared_w2,
           w1, w3, w2):
    args = [np.asarray(a) for a in (hidden_states, gate_w, bias, shared_w1,
                                    shared_w3, shared_w2, w1, w3, w2)]
    nc, in_maps, meta = _prepare(*args, MM_MODE)
    global LAST_RESULTS
    try:
        res = run_bass_kernel_spmd(nc, in_maps, list(range(NCORES)))
    except Exception:
        # transient NRT device errors happen; one retry clears them
        res = run_bass_kernel_spmd(nc, in_maps, list(range(NCORES)))
    LAST_RESULTS = res
    return _combine(res.results, meta)


# revision 39
# speedup vs baseline: 1.0090x; 1.0083x over previous
"""DeepSeekV3-style MoE layer on 8 Trainium2 NeuronCores.

Strategy (expert-parallel, host-side dispatch):
  - Host computes the sigmoid gate + top-2 routing (tiny: [8192,2048]@[2048,16]),
    gathers each expert's tokens. Experts are paired largest-with-smallest and
    sharded 2-per-core; per-slot capacities C0/C1 are the max count over the
    slot's 8 experts (exact, no rounding). The shared expert is data-parallel
    (1024 tokens per core).
  - Each core runs the same Bass/Tile program: 3 SwiGLU "units"
    (shared + 2 experts), weight-stationary matmuls at N<=512 in fp16
    (full PE rate, fast weight loads) with fp32 PSUM accumulation.
  - Phase 2 is h-major (stationary = w2 [128i,128h] tiles, moving = tokens):
    no ceil-128 token padding, exact token-column counts, output [H, n].
  - Startup: unit-0/it0 weights head the sync+scalar queues, x0 lands in
    four 256-column groups round-robined over all three DMA queues (gpsimd's
    share rides AHEAD of its w13 stream), and it0 runs 256-wide chunks so
    x delivery (~2.9us/group) outpaces PE consumption (~3.4us/chunk). PE
    stalls also reset the p-state clock ramp (0.65->1.2->2.4GHz over ~3us
    of continuous busy), so a stall-free startup pays twice.
    (All DMA is gated behind a fixed ~8.7us NEFF prologue; measured HW
    exec ~732us vs ~710us structural floor, 92.5% PE-active MFU.)
  - Gating scale is applied on-device during PSUM->SBUF evacuation via a
    host-replicated [128, cap] gate tile; host scatter-adds expert outputs
    back (transposing from [H, n]) and adds the shared output.

Layouts (host-prepared so every DMA is wide & contiguous):
  x*T   [16,128,n]        tokens transposed, h-tile major
  w1p   [3,11,128,2048]   phase-1 lhsT packs: [u][it][p=h%128][ht*128+j(=i%128)]
  w3p   same
  w2t   [3,11,128,2048]   w2 transposed: [u][it][p=i%128][h]
  gr*   [128,cap]         per-token gating scale, replicated over partitions
Outputs (h-major): ys [2048,1024], ye0 [2048,C0], ye1 [2048,C1] (fp32).
"""

import os
import sys

import numpy as np

if "/opt/trn_rl_repo" not in sys.path:
    sys.path.insert(0, "/opt/trn_rl_repo")

import concourse.bass as bass
import concourse.bacc as bacc
import concourse.mybir as mybir
import concourse.tile as tile
from concourse.bass_utils import run_bass_kernel_spmd

B, S, H, I, E, TOPK = 4, 2048, 2048, 1408, 16, 2
T = B * S               # 8192 tokens
NCORES = 8
NS = T // NCORES        # shared-expert tokens per core
HT, IT = H // 128, I // 128   # 16, 11
EPC = E // NCORES       # experts per core = 2

MM_MODE = os.environ.get("MOE_MM_MODE", "fp16")   # "fp16" | "f32r" | "bf16" | "f32"

LAST_RESULTS = None     # BassKernelResults of the last run (for test harness)

_PROGRAM_CACHE = {}
_PACK_CACHE = {}


def _sigmoid(x):
    out = np.empty_like(x)
    np.negative(x, out=out)
    np.exp(out, out=out)
    out += 1.0
    np.reciprocal(out, out=out)
    return out


def _chunks(n):
    """Split n into chunks <=512, all >=256 when n allows (f32r matmul runs
    at 1/4 rate below a 256-wide moving dim)."""
    out, rem = [], n
    while rem > 0:
        if rem <= 512:
            c = rem
        elif rem >= 768:
            c = 512
        else:  # rem in (512, 768): split so both pieces are >= 256
            c = rem - 256
        out.append(c)
        rem -= c
    return out


def _ramp_chunks(n):
    """Startup chunk widths for unit 0 / it 0, sized so the PE starts once
    ~2MB has landed and then NEVER stalls (x delivery outpaces consumption).
    Stall-free matters doubly: every PE idle gap resets the p-state clock
    ramp (0.65->2.4GHz over 3us of continuous busy)."""
    if n >= 1024:
        return [256, 256, 256, 256] + _chunks(n - 1024)
    return _chunks(n)


def _build_program(caps, mode):
    """caps = (C0, C1): exact token capacity of the two local expert slots."""
    key = (caps, mode)
    if key in _PROGRAM_CACHE:
        return _PROGRAM_CACHE[key]

    if mode == "bf16":
        in_dt = mybir.dt.bfloat16
    elif mode == "fp16":
        in_dt = mybir.dt.float16
    elif mode == "f32r":
        in_dt = mybir.dt.float32r
    else:
        in_dt = mybir.dt.float32
    f32 = mybir.dt.float32
    n_units = [NS, caps[0], caps[1]]
    CW = max(n_units)       # tile width shared by xt/g tags

    nc = bacc.Bacc("TRN2", target_bir_lowering=False, debug=False)

    xT = [nc.dram_tensor(f"x{u}T", [HT, 128, n_units[u]], in_dt,
                         kind="ExternalInput").ap() for u in range(3)]
    w1p = nc.dram_tensor("w1p", [3, IT, 128, H], in_dt, kind="ExternalInput").ap()
    w3p = nc.dram_tensor("w3p", [3, IT, 128, H], in_dt, kind="ExternalInput").ap()
    w2t = nc.dram_tensor("w2t", [3, IT, 128, H], in_dt, kind="ExternalInput").ap()
    gr = [None] + [nc.dram_tensor(f"gr{u}", [128, n_units[u]], f32,
                                  kind="ExternalInput").ap() for u in (1, 2)]
    yo = [nc.dram_tensor(["ys", "ye0", "ye1"][u], [H, n_units[u]], f32,
                         kind="ExternalOutput").ap() for u in range(3)]

    # DMA-queue plan (one HW queue per engine, FIFO): sync carries only the
    # activation loads, scalar only the output writes (plus half of the x0
    # race), gpsimd all weight streams. Emission order = descriptor order,
    # so prefetches are hoisted ahead of the compute that needs them.
    with tile.TileContext(nc) as tc:
        with (
            tc.tile_pool(name="xt", bufs=HT) as xt_pool,
            tc.tile_pool(name="g", bufs=IT + 1) as g_pool,
            tc.tile_pool(name="w13", bufs=6) as w13_pool,
            tc.tile_pool(name="w2", bufs=IT + 1) as w2_pool,
            tc.tile_pool(name="grb", bufs=2) as gr_pool,
            tc.tile_pool(name="ot", bufs=4) as out_pool,
            tc.tile_pool(name="ps", bufs=8, space="PSUM") as ps_pool,
        ):
            def load_xt(u):
                n_u = n_units[u]
                xts = [xt_pool.tile([128, CW], in_dt, tag="xt",
                                    name=f"xt{u}_{ht}") for ht in range(HT)]
                if u == 0:
                    # racing the kernel start: three queues, landing column
                    # groups that match the it0 ramp chunks [128,384,512,...]
                    # so each chunk's chains can begin while the rest streams
                    bounds = [0]
                    for w in _ramp_chunks(n_u):
                        bounds.append(bounds[-1] + w)
                    # merge the tail groups (beyond the ramp) into one DMA
                    if len(bounds) > 5:
                        bounds = bounds[:5] + [n_u]
                    # gpsimd carries a third of x AHEAD of its w13 stream,
                    # giving x strict priority on all three DMA queues
                    engs = [nc.sync, nc.scalar, nc.gpsimd]
                    for gi, (g0, g1) in enumerate(zip(bounds[:-1],
                                                      bounds[1:])):
                        for ht in range(HT):
                            engs[ht % 3].dma_start(out=xts[ht][:, g0:g1],
                                                   in_=xT[u][ht][:, g0:g1])
                    # it1 weights follow the x race on sync/scalar: they
                    # land right when it0's chains finish, and keep gpsimd
                    # free to deliver it2+ during it0
                    nc.sync.dma_start(out=it1_w[0][:], in_=w1p[0, 1])
                    nc.scalar.dma_start(out=it1_w[1][:], in_=w3p[0, 1])

                else:
                    for ht in range(HT):
                        nc.sync.dma_start(out=xts[ht][:, :n_u], in_=xT[u][ht])
                return xts

            # unit-0 it0 weights head the sync/scalar queues (before the x
            # race) so the first phase-1 chain can start ~4us after the
            # NEFF prologue; the x groups follow on three queues.
            w1t00 = w13_pool.tile([128, H], in_dt, tag="w13", name="w1t0_0")
            w3t00 = w13_pool.tile([128, H], in_dt, tag="w13", name="w3t0_0")
            nc.sync.dma_start(out=w1t00[:], in_=w1p[0, 0])
            nc.scalar.dma_start(out=w3t00[:], in_=w3p[0, 0])
            it1_w = (w13_pool.tile([128, H], in_dt, tag="w13", name="w1t0_1"),
                     w13_pool.tile([128, H], in_dt, tag="w13", name="w3t0_1"))
            xts = load_xt(0)
            for u in range(3):
                n_u = n_units[u]

                # ---- weight-stream emission (gpsimd): w13 it0/it1 first,
                # then this unit's gating tile, then the rest of w13, then
                # (after it10) the full w2 tile set for phase 2 (it lands
                # during this unit's phase 1; slot-waits pace the queue)
                w13s = []
                for it in range(IT):
                    if u == 0 and it == 0:
                        w13s.append((w1t00, w3t00))
                        continue
                    if u == 0 and it == 1:
                        w13s.append(it1_w)
                        continue

                    w1t = w13_pool.tile([128, H], in_dt, tag="w13",
                                        name=f"w1t{u}_{it}")
                    w3t = w13_pool.tile([128, H], in_dt, tag="w13",
                                        name=f"w3t{u}_{it}")
                    w13s.append((w1t, w3t))
                    nc.gpsimd.dma_start(out=w1t[:], in_=w1p[u, it])
                    nc.gpsimd.dma_start(out=w3t[:], in_=w3p[u, it])
                    if it == 5 and u > 0:
                        grt = gr_pool.tile([128, n_u], f32, tag="grb",
                                           name=f"grt{u}")
                        nc.gpsimd.dma_start(out=grt[:], in_=gr[u])
                w2s = []
                for it in range(IT):
                    w2tile = w2_pool.tile([128, H], in_dt, tag="w2",
                                          name=f"w2_{u}_{it}")
                    nc.gpsimd.dma_start(out=w2tile[:], in_=w2t[u, it])
                    w2s.append(w2tile)

                # ---- phase 1: G^T[i, t] = silu(W1 xT) * (W3 xT) ----
                gts = []
                for it in range(IT):
                    w1t, w3t = w13s[it]
                    gt = g_pool.tile([128, CW], in_dt, tag="g", name=f"g{u}_{it}")
                    gts.append(gt)
                    c0 = 0
                    cl = _ramp_chunks(n_u) if (u == 0 and it == 0) \
                        else _chunks(n_u)
                    for w in cl:
                        ps1 = ps_pool.tile([128, 512], f32, tag="ps",
                                           name=f"ps1_{u}_{it}_{c0}")
                        ps3 = ps_pool.tile([128, 512], f32, tag="ps",
                                           name=f"ps3_{u}_{it}_{c0}")
                        for ht in range(HT):
                            nc.tensor.matmul(
                                ps1[:, :w], w1t[:, ht * 128:(ht + 1) * 128],
                                xts[ht][:, c0:c0 + w],
                                start=(ht == 0), stop=(ht == HT - 1))
                        for ht in range(HT):
                            nc.tensor.matmul(
                                ps3[:, :w], w3t[:, ht * 128:(ht + 1) * 128],
                                xts[ht][:, c0:c0 + w],
                                start=(ht == 0), stop=(ht == HT - 1))
                        # silu(h1)*h3 = sigmoid(h1)*h1*h3 (Silu not in CoreSim)
                        gsl = gt[:, c0:c0 + w]
                        nc.scalar.activation(gsl, ps1[:, :w],
                                             mybir.ActivationFunctionType.Sigmoid)
                        nc.vector.tensor_mul(gsl, gsl, ps1[:, :w])
                        nc.vector.tensor_mul(gsl, gsl, ps3[:, :w])
                        c0 += w

                # next unit's activations stream during phase 2
                if u < 2:
                    next_xts = load_xt(u + 1)

                # ---- phase 2 (h-major): Y[h, t] = W2^T.T @ G^T, +gating ----
                # stationary = w2 [128i,128h] slices, moving = token columns;
                # exact token counts (no ceil-128 padding), output [H, n_u].
                p2c = _chunks(n_u)
                c0 = 0
                for ci, w in enumerate(p2c):
                    last_chunk = (u == 2 and ci == len(p2c) - 1)
                    for ht in range(HT):
                        ps = ps_pool.tile([128, 512], f32, tag="ps",
                                          name=f"ps2_{u}_{c0}_{ht}")
                        for it in range(IT):
                            nc.tensor.matmul(
                                ps[:, :w],
                                w2s[it][:, ht * 128:(ht + 1) * 128],
                                gts[it][:, c0:c0 + w],
                                start=(it == 0), stop=(it == IT - 1))
                        ot = out_pool.tile([128, 512], f32, tag="ot",
                                           name=f"ot{u}_{c0}_{ht}")
                        if u == 0:
                            nc.vector.tensor_copy(ot[:, :w], ps[:, :w])
                        else:
                            nc.vector.tensor_mul(ot[:, :w], ps[:, :w],
                                                 grt[:, c0:c0 + w])
                        # the very last chunk's writes alternate with the
                        # (by then idle) sync queue to halve the end drain
                        weng = nc.sync if (last_chunk and ht % 2) else nc.scalar
                        weng.dma_start(
                            out=yo[u][ht * 128:(ht + 1) * 128, c0:c0 + w],
                            in_=ot[:, :w])
                    c0 += w
                if u < 2:
                    xts = next_xts

    nc.compile()
    _PROGRAM_CACHE[key] = nc
    return nc


def _np_dt(mode):
    if mode == "bf16":
        import ml_dtypes
        return np.dtype(ml_dtypes.bfloat16)
    if mode == "fp16":
        return np.dtype(np.float16)
    return np.dtype(np.float32)


def _pack_w13(w, dt):
    """[I,H] -> [IT,128,H] with [it, p, ht*128+j] = w[it*128+j, ht*128+p]."""
    a = np.ascontiguousarray(
        w.reshape(IT, 128, HT, 128).transpose(0, 3, 2, 1), dtype=dt)
    return a.reshape(IT, 128, H)


def _pack_w2(w, dt):
    """[H,I] -> [IT,128,H]  (= w.T tiled along I)."""
    return np.ascontiguousarray(w.T.reshape(IT, 128, H), dtype=dt)


def _pack_all_weights(shared_w1, shared_w3, shared_w2, w1, w3, w2, mode):
    key = (id(w1), id(w2), id(w3), mode)
    if _PACK_CACHE.get("key") == key:
        return _PACK_CACHE["val"]
    dt = _np_dt(mode)
    p1 = [_pack_w13(shared_w1, dt)] + [_pack_w13(w1[e], dt) for e in range(E)]
    p3 = [_pack_w13(shared_w3, dt)] + [_pack_w13(w3[e], dt) for e in range(E)]
    p2 = [_pack_w2(shared_w2, dt)] + [_pack_w2(w2[e], dt) for e in range(E)]
    val = (p1, p3, p2)
    _PACK_CACHE["key"] = key
    _PACK_CACHE["val"] = val
    return val


def _prepare(hidden_states, gate_w, bias, shared_w1, shared_w3, shared_w2,
             w1, w3, w2, mode):
    """Host routing + per-core input maps. Returns (nc, in_maps, meta)."""
    x = np.ascontiguousarray(hidden_states.reshape(T, H), dtype=np.float32)

    scores = _sigmoid(x @ gate_w.T.astype(np.float32))
    routing = scores + bias.astype(np.float32)[None, :]
    topk = np.argsort(-routing, axis=1, kind="stable")[:, :TOPK]
    sel = np.take_along_axis(scores, topk, axis=1)
    gating = (sel / sel.sum(axis=1, keepdims=True)).astype(np.float32)

    flat_t = np.repeat(np.arange(T), TOPK)
    flat_e = topk.ravel()
    flat_g = gating.ravel()
    order = np.argsort(flat_e, kind="stable")
    flat_t, flat_g = flat_t[order], flat_g[order]
    counts = np.bincount(flat_e, minlength=E)
    offs = np.zeros(E + 1, np.int64)
    np.cumsum(counts, out=offs[1:])

    # pair largest with smallest: slot0 = rank c, slot1 = rank 15-c
    # (minimizes C0+C1 = c(1)+c(9), which is optimal for a 2-slot SPMD plan)
    rank = np.argsort(-counts, kind="stable")
    slot_experts = [(int(rank[c]), int(rank[E - 1 - c])) for c in range(NCORES)]
    C0 = max(1, int(counts[rank[0]]))
    C1 = max(1, int(counts[rank[NCORES]]))
    caps = (C0, C1)

    nc = _build_program(caps, mode)
    dt = _np_dt(mode)

    p1, p3, p2 = _pack_all_weights(shared_w1, shared_w3, shared_w2,
                                   w1, w3, w2, mode)
    xc = x.astype(dt, copy=False)

    tok_ids = []
    in_maps = []
    for c in range(NCORES):
        im = {"x0T": np.ascontiguousarray(
            xc[c * NS:(c + 1) * NS].T).reshape(HT, 128, NS)}
        ids_pair = []
        for j, e in enumerate(slot_experts[c]):
            Cj = caps[j]
            ids = flat_t[offs[e]:offs[e + 1]]
            ids_pair.append(ids)
            n = len(ids)
            xg = np.zeros((Cj, H), dt)
            xg[:n] = xc[ids]
            im[f"x{j + 1}T"] = np.ascontiguousarray(xg.T).reshape(HT, 128, Cj)
            grow = np.zeros((Cj,), np.float32)
            grow[:n] = flat_g[offs[e]:offs[e + 1]]
            im[f"gr{j + 1}"] = np.ascontiguousarray(
                np.broadcast_to(grow, (128, Cj)))
        e0, e1 = slot_experts[c]
        im["w1p"] = np.stack([p1[0], p1[1 + e0], p1[1 + e1]])
        im["w3p"] = np.stack([p3[0], p3[1 + e0], p3[1 + e1]])
        im["w2t"] = np.stack([p2[0], p2[1 + e0], p2[1 + e1]])
        tok_ids.append(ids_pair)
        in_maps.append(im)

    meta = {"counts": counts, "tok_ids": tok_ids, "slot_experts": slot_experts,
            "caps": caps, "shape": hidden_states.shape}
    return nc, in_maps, meta


def _combine(results, meta):
    out = np.empty((T, H), np.float32)
    for c in range(NCORES):
        out[c * NS:(c + 1) * NS] = results[c]["ys"].T
    for c in range(NCORES):
        for j in range(EPC):
            ids = meta["tok_ids"][c][j]
            out[ids] += results[c][f"ye{j}"][:, :len(ids)].T
    return out.reshape(meta["shape"])


def kernel(hidden_states, gate_w, bias, shared_w1, shared_w3, shared_w2,
           w1, w3, w2):
    args = [np.asarray(a) for a in (hidden_states, gate_w, bias, shared_w1,
                                    shared_w3, shared_w2, w1, w3, w2)]
    nc, in_maps, meta = _prepare(*args, MM_MODE)
    global LAST_RESULTS
    try:
        res = run_bass_kernel_spmd(nc, in_maps, list(range(NCORES)))
    except Exception:
        # transient NRT device errors happen; one retry clears them
        res = run_bass_kernel_spmd(nc, in_maps, list(range(NCORES)))
    LAST_RESULTS = res
    return _combine(res.results, meta)
